# revision 1
# baseline (speedup 1.0000x reference)
"""DGCRN Trainium2 Bass kernel.

Problem: nn_DGCRN_67327907332247 (B=32, T=12, N=512, DIN=2, HID=64, CHEB_K=3,
EMB=10, DOUT=1, YCOV=1). Data-parallel over batch: 8 cores x 4 batches each.

Design notes (feature-major layout):
 - Activations are stored transposed in SBUF: h^T is [64, 512] (features on
   partitions, nodes on the free dim). The GRU dense matmuls then use the
   weight matrices as-stored for lhsT (no activation transposes for them):
       z_r^T[128,512] = sum_k gW_k[66,128].T @ x_g_k^T[66,512]
 - Graph propagation u = A @ x needs node-contraction, so the activation is
   transposed on the PE (4 [66,128] -> [128,66] transposes into one PSUM tile)
   and then u^T[66,512] = inp_nodemajor[128,66].T @ A^T[128,512] accumulated
   over 4 node tiles.
 - Chebyshev: u2 = (2A^2 - I) @ x = 2*A@(A@x) - x, so A^2 is never formed
   (saves 32*512^3 MACs and 33MB).  The "-x" and "*2" fold into one fused
   scalar_tensor_tensor DVE op off PSUM.
 - Supports: A^T tiles stay SBUF-resident: encoder 4x[128,512], decoder
   4 batches x 4x[128,512] (4MB/core).
 - Softmax(relu(S), rows): row-max subtraction is skipped (relu bounds the
   argument; exp stays well inside fp32 range), exp's row-sum is fused into
   the ACT op via accum_out.
 - All non-transpose matmuls run in float32r (single-pass fp32, 4x the fp32
   rate); every producer of a matmul operand writes through a .bitcast(f32r)
   AP so the BIR verifier's "rounded producer" rule is satisfied (DMA loads
   are staged + round-copied; zero-init is a round-copy from a memset tile).
 - Feature layout inside state tiles is [h(0:64); xt(64:66)] (weight rows
   permuted to match) so matmul/elementwise base-partition rules (0/32/64)
   hold everywhere; h state ping-pongs between two tiles per batch.
"""

import os
import time

import numpy as np

B = 32
NCORES = 8
BL = B // NCORES  # 4 local batches
T = 12
N = 512
NT = N // 128  # 4 node tiles
DIN = 2
HID = 64
EMB = 10
CIN = DIN + HID  # 66
K = 3

_CACHE = {}

# scheduling knobs (model-sweep via env)
WP_BUFS = int(os.environ.get("K_WP_BUFS", "3"))
PS_TP = int(os.environ.get("K_PS_TP", "2"))
PS_MMU = int(os.environ.get("K_PS_MMU", "3"))
PS_MMZ = int(os.environ.get("K_PS_MMZ", "3"))
PS_APS = int(os.environ.get("K_PS_APS", "1"))


def _build_module():
    import concourse.bacc as bacc
    import concourse.mybir as mybir
    from concourse import masks, tile

    f32 = mybir.dt.float32
    Alu = mybir.AluOpType
    Act = mybir.ActivationFunctionType
    f32r = mybir.dt.float32r

    nc = bacc.Bacc("TRN2", target_bir_lowering=False, debug=False)

    def mm(out, lhsT, rhs, **kw):
        # float32r: single-pass fp32 matmul, 4x faster than fp32 for N>=256.
        # Operands must be produced pre-rounded to f32r (BIR verifier rule);
        # producers write through rr() below.
        nc.tensor.matmul(out, lhsT.bitcast(f32r), rhs.bitcast(f32r), **kw)

    def rr(ap):
        return ap.bitcast(f32r)

    x_d = nc.dram_tensor("x", [BL, T, N, DIN], f32, kind="ExternalInput").ap()
    y_d = nc.dram_tensor("y_cov", [BL, T, N, 1], f32, kind="ExternalInput").ap()
    ne_d = nc.dram_tensor("node_emb", [N, EMB], f32, kind="ExternalInput").ap()
    egw_d = nc.dram_tensor("enc_gW", [K * CIN, 2 * HID], f32, kind="ExternalInput").ap()
    egb_d = nc.dram_tensor("enc_gb", [2 * HID], f32, kind="ExternalInput").ap()
    euw_d = nc.dram_tensor("enc_uW", [K * CIN, HID], f32, kind="ExternalInput").ap()
    eub_d = nc.dram_tensor("enc_ub", [HID], f32, kind="ExternalInput").ap()
    dgw_d = nc.dram_tensor("dec_gW", [K * CIN, 2 * HID], f32, kind="ExternalInput").ap()
    dgb_d = nc.dram_tensor("dec_gb", [2 * HID], f32, kind="ExternalInput").ap()
    duw_d = nc.dram_tensor("dec_uW", [K * CIN, HID], f32, kind="ExternalInput").ap()
    dub_d = nc.dram_tensor("dec_ub", [HID], f32, kind="ExternalInput").ap()
    pw_d = nc.dram_tensor("proj_W", [HID, 1], f32, kind="ExternalInput").ap()
    pb_d = nc.dram_tensor("proj_b", [1], f32, kind="ExternalInput").ap()
    hw_d = nc.dram_tensor("hyper_W", [HID, EMB], f32, kind="ExternalInput").ap()
    hb_d = nc.dram_tensor("hyper_b", [EMB], f32, kind="ExternalInput").ap()
    out_d = nc.dram_tensor("out", [BL, T, N, 1], f32, kind="ExternalOutput").ap()

    with tile.TileContext(nc) as tc:
        with (
            tc.tile_pool(name="const", bufs=1) as cp,
            tc.tile_pool(name="state", bufs=1) as sp,
            tc.tile_pool(name="work", bufs=WP_BUFS) as wp,
            tc.tile_pool(name="psum", bufs=1, space="PSUM") as pp,
        ):
            ident = cp.tile([128, 128], f32)
            masks.make_identity(nc, ident[:])
            # id slab per node-tile: zeros except I at column block j
            idsl = []
            for j in range(NT):
                s = cp.tile([128, N], f32, name=f"idsl{j}")
                nc.vector.memset(s[:], 0.0)
                nc.vector.tensor_copy(s[:, j * 128 : (j + 1) * 128], ident[:])
                idsl.append(s)

            # ---- weights (DMA into scratch, round-copy into final f32r) ----
            def wslices(name, src, cols, hscale=1.0):
                # feature layout is [h(0:64); xt(64:66)] -> permute W rows.
                # hscale pre-scales the h-feature rows (cand carries 2*z*h
                # when sigmoid is computed as (tanh(x/2)+1)/2, so dec/enc uW
                # h-rows absorb the 1/2).
                ts = [cp.tile([CIN, cols], f32, name=f"{name}{k}") for k in range(K)]
                for k in range(K):
                    ws = wp.tile([CIN, cols], f32, name=f"ws_{name}{k}", tag="wstg")
                    nc.sync.dma_start(
                        ws[0:HID, :], src[k * CIN + DIN : (k + 1) * CIN, :]
                    )
                    nc.sync.dma_start(
                        ws[HID:CIN, :], src[k * CIN : k * CIN + DIN, :]
                    )
                    nc.scalar.mul(rr(ts[k][0:HID, :]), ws[0:HID, :], hscale)
                    nc.scalar.copy(rr(ts[k][HID:CIN, :]), ws[HID:CIN, :])
                return ts

            gw_e = wslices("gw_e", egw_d, 2 * HID)
            uw_e = wslices("uw_e", euw_d, HID)
            gw_dc = wslices("gw_dc", dgw_d, 2 * HID)
            uw_dc = wslices("uw_dc", duw_d, HID)

            def bias_tile(name, src, n, scale=1.0):
                t = cp.tile([n, 1], f32, name=name)
                bs = wp.tile([n, 1], f32, name=f"bs_{name}", tag="wstg2")
                nc.sync.dma_start(bs[:], src.rearrange("(p o) -> p o", o=1))
                nc.scalar.mul(t[:], bs[:], scale)
                return t

            gbz_e = bias_tile("gbz_e", egb_d[0:HID], HID)
            gbr_e = bias_tile("gbr_e", egb_d[HID : 2 * HID], HID)
            ub_e = bias_tile("ub_e", eub_d, HID)
            gbz_dc = bias_tile("gbz_dc", dgb_d[0:HID], HID)
            gbr_dc = bias_tile("gbr_dc", dgb_d[HID : 2 * HID], HID)
            ub_dc = bias_tile("ub_dc", dub_d, HID)
            pb = bias_tile("pb", pb_d, 1)
            hb = bias_tile("hb", hb_d, EMB)
            pw = cp.tile([HID, 1], f32)
            pws = wp.tile([HID, 1], f32, tag="wstg2")
            nc.sync.dma_start(pws[:], pw_d[:, :])
            nc.scalar.copy(rr(pw[:]), pws[:])
            hwt = cp.tile([HID, EMB], f32)
            hws = wp.tile([HID, EMB], f32, tag="wstg2")
            nc.sync.dma_start(hws[:], hw_d[:, :])
            nc.scalar.copy(rr(hwt[:]), hws[:])

            # staging for xt rows, partitions 64:66 (lane-aligned round-copy);
            # fresh tile per cell so input DMAs prefetch ahead of the chain
            def xstg_tile(tag):
                return wp.tile([CIN, N], f32, name=f"xs_{tag}", tag="xstg", bufs=3)

            # go/ycov staging must persist across decoder steps (go_t feeds t+1)
            gstg = [sp.tile([CIN, N], f32, name=f"gstg{b}") for b in range(BL)]

            # ---- persistent per-batch state (ping-pong double buffer) ----
            # inp [66,512]: rows 0:64 h^T, rows 64:66 = xt (enc) / go,ycov (dec)
            zstg = cp.tile([CIN, N], f32, name="zstg")
            nc.vector.memset(zstg[:], 0.0)
            inp_pp = []
            cand_t = []
            for b in range(BL):
                pair = []
                for p in range(2):
                    it = sp.tile([CIN, N], f32, name=f"inp{b}_{p}")
                    # f32r memset is invalid ISA; round-copy zeros instead
                    nc.vector.tensor_copy(rr(it[:]), zstg[:])
                    pair.append(it)
                inp_pp.append(pair)
                ct = sp.tile([CIN, N], f32, name=f"cand{b}")
                cand_t.append(ct)

            # ---- support construction helper ----
            # emit_scores(i) -> PSUM tile [128,512] with raw scores rows
            # i*128..  Returns 4 SBUF tiles [128,512] of A^T (node-major
            # column tiles).
            def build_support(emit_scores, at_tiles, at2_tiles, tag):
                a_nm = []
                for i in range(NT):
                    ps_s = emit_scores(i)
                    e_in = wp.tile([128, N], f32, name=f"e_in_{tag}{i}", tag="e_in")
                    nc.vector.tensor_scalar_max(e_in[:], ps_s[:], 0.0)
                    e_x = wp.tile(
                        [128, N], f32, name=f"e_x_{tag}{i}", tag="e_x", bufs=NT
                    )
                    esum = wp.tile([128, 1], f32, name=f"esum_{tag}{i}", tag="esum")
                    nc.scalar.activation(
                        e_x[:], e_in[:], Act.Exp, accum_out=esum[:]
                    )
                    rinv = wp.tile([128, 1], f32, name=f"rinv_{tag}{i}", tag="rinv")
                    nc.vector.reciprocal(rinv[:], esum[:])
                    an = wp.tile([128, N], f32, name=f"an_{tag}{i}", tag="anm", bufs=NT)
                    nc.vector.tensor_scalar_mul(rr(an[:]), e_x[:], rinv[:])
                    a_nm.append(an)
                for j in range(NT):
                    ps_t = pp.tile([128, N], f32, name=f"ps_t_{tag}{j}", tag="tp", bufs=PS_TP)
                    for i in range(NT):
                        nc.tensor.matmul(
                            ps_t[:, i * 128 : (i + 1) * 128],
                            a_nm[i][:, j * 128 : (j + 1) * 128],
                            ident[:],
                            is_transpose=True,
                            skip_group_check=True,
                        )
                    nc.scalar.copy(rr(at_tiles[j][:]), ps_t[:])
                # A2T[j] = 2*(A^T A^T)[j-rows] - I = ((2A^2 - I)^T)[j-rows]
                for j in range(NT):
                    ps_c = pp.tile(
                        [128, N], f32, name=f"ps_c_{tag}{j}", tag="mmu", bufs=PS_MMU
                    )
                    for k in range(NT):
                        mm(
                            ps_c[:],
                            a_nm[k][:, j * 128 : (j + 1) * 128],
                            at_tiles[k][:],
                            start=(k == 0),
                            stop=(k == NT - 1),
                        )
                    nc.vector.scalar_tensor_tensor(
                        rr(at2_tiles[j][:]), ps_c[:], 2.0, idsl[j][:], Alu.mult,
                        Alu.subtract,
                    )
                return

            # ---- encoder support ----
            ne_nm = []
            for i in range(NT):
                t = wp.tile([128, EMB], f32, name=f"ne_nm{i}", tag="ne_nm", bufs=NT)
                nc.sync.dma_start(t[:], ne_d[i * 128 : (i + 1) * 128, :])
                ne_nm.append(t)
            ps_ne = pp.tile([EMB, N], f32, tag="mmu", bufs=PS_MMU)
            for i in range(NT):
                nc.tensor.matmul(
                    ps_ne[:, i * 128 : (i + 1) * 128],
                    ne_nm[i][:],
                    ident[:],
                    is_transpose=True,
                    skip_group_check=True,
                )
            neT = cp.tile([EMB, N], f32)
            nc.vector.tensor_copy(rr(neT[:]), ps_ne[:])

            aet = [cp.tile([128, N], f32, name=f"aet{j}") for j in range(NT)]
            aet2 = [cp.tile([128, N], f32, name=f"aet2_{j}") for j in range(NT)]

            def enc_scores(i):
                ps = pp.tile([128, N], f32, name=f"ps_enc_s{i}", tag="mmz", bufs=PS_MMZ)
                mm(
                    ps[:], neT[:, i * 128 : (i + 1) * 128], neT[:], start=True, stop=True
                )
                return ps

            build_support(enc_scores, aet, aet2, "enc")

            # ---- the GRU cell ----
            def graph_conv(src, at, at2, tag, bufs=2):
                """src [66,512] feature-major -> (xg1, xg2) [66,512] SBUF."""
                tp = pp.tile([128, NT * CIN], f32, name=f"tp_{tag}", tag="tp", bufs=PS_TP)
                for j in range(NT):
                    nc.tensor.matmul(
                        tp[:, j * CIN : (j + 1) * CIN],
                        src[:, j * 128 : (j + 1) * 128],
                        ident[0:CIN, 0:CIN],
                        is_transpose=True,
                        skip_group_check=True,
                    )
                srcN = wp.tile([128, NT * CIN], f32, name=f"srcN_{tag}", tag="srcN")
                nc.vector.tensor_copy(rr(srcN[:]), tp[:])
                ps1 = pp.tile([CIN, N], f32, name=f"ps1_{tag}", tag="mmu", bufs=PS_MMU)
                for j in range(NT):
                    mm(
                        ps1[:],
                        srcN[:, j * CIN : (j + 1) * CIN],
                        at[j][:],
                        start=(j == 0),
                        stop=(j == NT - 1),
                    )
                xg1 = wp.tile([CIN, N], f32, name=f"xg1_{tag}", tag="xg1", bufs=bufs)
                nc.scalar.copy(rr(xg1[:]), ps1[:])
                ps2 = pp.tile([CIN, N], f32, name=f"ps2_{tag}", tag="mmu", bufs=PS_MMU)
                for j in range(NT):
                    mm(
                        ps2[:],
                        srcN[:, j * CIN : (j + 1) * CIN],
                        at2[j][:],
                        start=(j == 0),
                        stop=(j == NT - 1),
                    )
                xg2 = wp.tile([CIN, N], f32, name=f"xg2_{tag}", tag="xg2", bufs=bufs)
                nc.vector.tensor_copy(rr(xg2[:]), ps2[:])
                return xg1, xg2

            def cell(b, inp, inp_nx, at, at2, gw, gb, uw, ub, tag):
                cand = cand_t[b]
                gbz, gbr = gb
                g1, g2 = graph_conv(inp, at, at2, f"g{tag}")
                psz = pp.tile([HID, N], f32, name=f"psz_{tag}", tag="mmz", bufs=PS_MMZ)
                psr = pp.tile([HID, N], f32, name=f"psr_{tag}", tag="mmz", bufs=PS_MMZ)
                for k, src in enumerate((inp, g1, g2)):
                    mm(
                        psz[:],
                        gw[k][:, 0:HID],
                        src[:],
                        start=(k == 0),
                        stop=(k == K - 1),
                    )
                for k, src in enumerate((inp, g1, g2)):
                    mm(
                        psr[:],
                        gw[k][:, HID : 2 * HID],
                        src[:],
                        start=(k == 0),
                        stop=(k == K - 1),
                    )
                z_t = wp.tile([HID, N], f32, name=f"z_{tag}", tag="z_t")
                nc.scalar.activation(z_t[:], psz[:], Act.Sigmoid, bias=gbz[:])
                r_t = wp.tile([HID, N], f32, name=f"r_{tag}", tag="r_t")
                nc.scalar.activation(r_t[:], psr[:], Act.Sigmoid, bias=gbr[:])
                nc.gpsimd.tensor_copy(rr(cand[HID:CIN, :]), inp[HID:CIN, :])
                nc.vector.tensor_mul(rr(cand[0:HID, :]), z_t[:], inp[0:HID, :])
                c1, c2 = graph_conv(cand, at, at2, f"c{tag}")
                psh = pp.tile([HID, N], f32, name=f"psh_{tag}", tag="mmz", bufs=PS_MMZ)
                for k, src in enumerate((cand, c1, c2)):
                    mm(
                        psh[:], uw[k][:], src[:], start=(k == 0), stop=(k == K - 1)
                    )
                hc = wp.tile([HID, N], f32, name=f"hc_{tag}", tag="hc")
                nc.scalar.activation(hc[:], psh[:], Act.Tanh, bias=ub[:])
                d = wp.tile([HID, N], f32, name=f"d_{tag}", tag="d")
                nc.gpsimd.tensor_sub(d[:], inp[0:HID, :], hc[:])
                rd = wp.tile([HID, N], f32, name=f"rd_{tag}", tag="rd")
                nc.gpsimd.tensor_mul(rd[:], r_t[:], d[:])
                nc.vector.tensor_add(rr(inp_nx[0:HID, :]), hc[:], rd[:])

            # ---- encoder ----
            for t in range(T):
                for b in range(BL):
                    cur, nxt = inp_pp[b][t % 2], inp_pp[b][(t + 1) % 2]
                    xs = xstg_tile(f"e{t}b{b}")
                    nc.sync.dma_start(
                        xs[HID:CIN, :], x_d[b, t].rearrange("n c -> c n")
                    )
                    nc.vector.tensor_copy(rr(cur[HID:CIN, :]), xs[HID:CIN, :])
                    cell(
                        b, cur, nxt, aet, aet2, gw_e, (gbz_e, gbr_e), uw_e,
                        ub_e, f"e{t}b{b}"
                    )

            # ---- decoder supports (hyper-network) ----
            adt = [
                [cp.tile([128, N], f32, name=f"adt{b}_{j}") for j in range(NT)]
                for b in range(BL)
            ]
            adt2 = [
                [cp.tile([128, N], f32, name=f"adt2_{b}_{j}") for j in range(NT)]
                for b in range(BL)
            ]
            for b in range(BL):
                ps_h = pp.tile([EMB, N], f32, name=f"ps_hyp{b}", tag="mmu", bufs=PS_MMU)
                mm(
                    ps_h[:], hwt[:], inp_pp[b][T % 2][0:HID, :], start=True, stop=True
                )
                neb = wp.tile([EMB, N], f32, name=f"neb{b}", tag="neb")
                nc.scalar.activation(rr(neb[:]), ps_h[:], Act.Identity, bias=hb[:])

                def dec_scores(i, neb=neb, b=b):
                    ps = pp.tile(
                        [128, N], f32, name=f"ps_dec_s{b}_{i}", tag="mmz", bufs=PS_MMZ
                    )
                    mm(
                        ps[:],
                        neb[:, i * 128 : (i + 1) * 128],
                        neb[:],
                        start=True,
                        stop=True,
                    )
                    return ps

                build_support(dec_scores, adt[b], adt2[b], f"dec{b}")

            # ---- decoder ----
            for b in range(BL):
                nc.vector.memset(gstg[b][HID : HID + 1, :], 0.0)  # go_0 = 0
            for t in range(T):
                for b in range(BL):
                    cur = inp_pp[b][(T + t) % 2]
                    nxt = inp_pp[b][(T + t + 1) % 2]
                    nc.sync.dma_start(
                        gstg[b][HID + 1 : CIN, :], y_d[b, t].rearrange("n c -> c n")
                    )
                    nc.vector.tensor_copy(
                        rr(cur[HID:CIN, :]), gstg[b][HID:CIN, :]
                    )
                    cell(
                        b, cur, nxt, adt[b], adt2[b], gw_dc, (gbz_dc, gbr_dc),
                        uw_dc, ub_dc, f"d{t}b{b}"
                    )
                    psg = pp.tile(
                        [1, N], f32, name=f"psg_d{t}b{b}", tag="mmu", bufs=PS_MMU
                    )
                    mm(psg[:], pw[:], nxt[0:HID, :], start=True, stop=True)
                    nc.scalar.activation(
                        gstg[b][0:1, :], psg[:], Act.Identity, bias=pb[:]
                    )
                    nc.sync.dma_start(
                        out_d[b, t].rearrange("n c -> c n"), gstg[b][0:1, :]
                    )
                    # relocate go to partition 64 for next step's round-copy
                    nc.sync.dma_start(
                        gstg[b][HID : HID + 1, :], gstg[b][0:1, :]
                    )

    nc.compile()
    return nc


def _get_module():
    if "nc" not in _CACHE:
        _CACHE["nc"] = _build_module()
    return _CACHE["nc"]


def _in_maps(inputs):
    shared = {
        k: np.ascontiguousarray(np.asarray(inputs[k], dtype=np.float32))
        for k in (
            "node_emb",
            "enc_gW",
            "enc_gb",
            "enc_uW",
            "enc_ub",
            "dec_gW",
            "dec_gb",
            "dec_uW",
            "dec_ub",
            "proj_W",
            "proj_b",
            "hyper_W",
            "hyper_b",
        )
    }
    x = np.ascontiguousarray(np.asarray(inputs["x"], dtype=np.float32))
    y = np.ascontiguousarray(np.asarray(inputs["y_cov"], dtype=np.float32))
    maps = []
    for c in range(NCORES):
        m = dict(shared)
        m["x"] = np.ascontiguousarray(x[c * BL : (c + 1) * BL])
        m["y_cov"] = np.ascontiguousarray(y[c * BL : (c + 1) * BL])
        maps.append(m)
    return maps


def kernel(**inputs) -> np.ndarray:
    from concourse.bass_utils import run_bass_kernel_spmd

    nc = _get_module()
    maps = _in_maps(inputs)
    res = run_bass_kernel_spmd(nc, maps, list(range(NCORES)))
    out = np.concatenate([res.results[c]["out"] for c in range(NCORES)], axis=0)
    return out.astype(np.float32)



# revision 11
# speedup vs baseline: 1.8392x; 1.8392x over previous
"""DGCRN Trainium2 Bass kernel (restructured).

Problem: nn_DGCRN_67327907332247 (B=32, T=12, N=512, DIN=2, HID=64, CHEB_K=3,
EMB=10, DOUT=1, YCOV=1). Data-parallel over batch: 8 cores x 4 batches each.

Design (v2 — "v-projection" formulation, batch-stage interleaving):
 - State feature-major: inp [66,512] = [h(0:64); xt/go,ycov(64:66)].
 - Per gate path, exploit linearity G_k(x) W_k = (x W_k) pre-projected:
     v_j = inp_nm[j] @ [W1|W2]   (4 matmuls, moving dim 256, node-major out)
     zr^T = W0^T@inp + sum_j v1_j^T@A^T_j + sum_j v2_j^T@A2^T_j  (9 mm, 512)
   This kills all per-cell PE transposes and the xg PSUM->SBUF round trips.
 - z/r gates fused into one [128,512] accumulation + one sigmoid (bias is the
   stacked [gbz;gbr]).
 - Work for the 4 local batches is emitted stage-interleaved (vb0 vb1 zr0 vb2
   zr1 ...) so the PE never drains; a continuously-busy PE ramps from 1.2GHz
   to 2.4GHz (p-state) which alone is ~2x on matmul time.
 - All matmuls f32r with moving dims >= 256 (1 cycle/row). Producers of
   matmul operands write through .bitcast(f32r) APs (BIR rounded-producer
   rule); DMA loads are staged + round-copied.
 - Supports: A^T / (2A^2-I)^T tiles SBUF-resident as in v1; softmax skips
   row-max (relu-bounded); -I handled by subtracting the [128,128] identity
   from the diagonal block (no idsl slabs).
 - Elementwise work spread across Scalar(ACT)/Vector(DVE)/GpSimd(Pool) to
   stay under the PE's per-cell time.
"""

import numpy as np

B = 32
NCORES = 8
BL = B // NCORES  # 4 local batches
T = 12
N = 512
NT = N // 128  # 4 node tiles
DIN = 2
HID = 64
EMB = 10
CIN = DIN + HID  # 66
K = 3

_CACHE = {}


def _build_module():
    import concourse.bacc as bacc
    import concourse.mybir as mybir
    from concourse import masks, tile

    f32 = mybir.dt.float32
    Act = mybir.ActivationFunctionType
    f32r = mybir.dt.float32r

    nc = bacc.Bacc("TRN2", target_bir_lowering=False, debug=False)

    def mm(out, lhsT, rhs, **kw):
        nc.tensor.matmul(out, lhsT.bitcast(f32r), rhs.bitcast(f32r), **kw)

    def rr(ap):
        return ap.bitcast(f32r)

    x_d = nc.dram_tensor("x", [BL, T, N, DIN], f32, kind="ExternalInput").ap()
    y_d = nc.dram_tensor("y_cov", [BL, T, N, 1], f32, kind="ExternalInput").ap()
    ne_d = nc.dram_tensor("node_emb", [N, EMB], f32, kind="ExternalInput").ap()
    egw_d = nc.dram_tensor("enc_gW", [K * CIN, 2 * HID], f32, kind="ExternalInput").ap()
    egb_d = nc.dram_tensor("enc_gb", [2 * HID], f32, kind="ExternalInput").ap()
    euw_d = nc.dram_tensor("enc_uW", [K * CIN, HID], f32, kind="ExternalInput").ap()
    eub_d = nc.dram_tensor("enc_ub", [HID], f32, kind="ExternalInput").ap()
    dgw_d = nc.dram_tensor("dec_gW", [K * CIN, 2 * HID], f32, kind="ExternalInput").ap()
    dgb_d = nc.dram_tensor("dec_gb", [2 * HID], f32, kind="ExternalInput").ap()
    duw_d = nc.dram_tensor("dec_uW", [K * CIN, HID], f32, kind="ExternalInput").ap()
    dub_d = nc.dram_tensor("dec_ub", [HID], f32, kind="ExternalInput").ap()
    pw_d = nc.dram_tensor("proj_W", [HID, 1], f32, kind="ExternalInput").ap()
    pb_d = nc.dram_tensor("proj_b", [1], f32, kind="ExternalInput").ap()
    hw_d = nc.dram_tensor("hyper_W", [HID, EMB], f32, kind="ExternalInput").ap()
    hb_d = nc.dram_tensor("hyper_b", [EMB], f32, kind="ExternalInput").ap()
    out_d = nc.dram_tensor("out", [BL, T, N, 1], f32, kind="ExternalOutput").ap()

    with tile.TileContext(nc) as tc:
        with (
            tc.tile_pool(name="const", bufs=1) as cp,
            tc.tile_pool(name="state", bufs=1) as sp,
            tc.tile_pool(name="work", bufs=2) as wp,
            tc.tile_pool(name="psum", bufs=1, space="PSUM") as pp,
        ):
            ident = cp.tile([128, 128], f32)
            masks.make_identity(nc, ident[:])

            def acc_tile(name, p=128, n=N):
                return pp.tile([p, n], f32, name=name, tag="acc", bufs=7)

            # ---- weights ----------------------------------------------------
            # feature layout [h(0:64); xt(64:66)] -> permute W rows.
            def wload(dst, src, k, c0, cols):
                ws = wp.tile([CIN, cols], f32, name=f"ws{k}_{c0}", tag="wstg")
                nc.sync.dma_start(ws[0:HID, :], src[k * CIN + DIN : (k + 1) * CIN, :])
                nc.sync.dma_start(ws[HID:CIN, :], src[k * CIN : k * CIN + DIN, :])
                nc.scalar.copy(rr(dst[:, c0 : c0 + cols]), ws[:])

            gw0_e = cp.tile([CIN, 2 * HID], f32)
            gws_e = cp.tile([CIN, 4 * HID], f32)
            uw0_e = cp.tile([CIN, HID], f32)
            uws_e = cp.tile([CIN, 4 * HID], f32)
            gw0_d = cp.tile([CIN, 2 * HID], f32)
            gws_d = cp.tile([CIN, 4 * HID], f32)
            uw0_d = cp.tile([CIN, HID], f32)
            uws_d = cp.tile([CIN, 4 * HID], f32)
            wload(gw0_e, egw_d, 0, 0, 2 * HID)
            wload(gws_e, egw_d, 1, 0, 2 * HID)
            wload(gws_e, egw_d, 2, 2 * HID, 2 * HID)
            wload(uw0_e, euw_d, 0, 0, HID)
            wload(uws_e, euw_d, 1, 0, HID)
            wload(uws_e, euw_d, 2, HID, HID)
            wload(gw0_d, dgw_d, 0, 0, 2 * HID)
            wload(gws_d, dgw_d, 1, 0, 2 * HID)
            wload(gws_d, dgw_d, 2, 2 * HID, 2 * HID)
            wload(uw0_d, duw_d, 0, 0, HID)
            wload(uws_d, duw_d, 1, 0, HID)
            wload(uws_d, duw_d, 2, HID, HID)
            # zero-pad cols 128:256 of the uW stacks (keeps moving dim 256)
            zpad = wp.tile([CIN, 2 * HID], f32, tag="wstg")
            nc.vector.memset(zpad[:], 0.0)
            nc.scalar.copy(rr(uws_e[:, 2 * HID : 4 * HID]), zpad[:])
            nc.scalar.copy(rr(uws_d[:, 2 * HID : 4 * HID]), zpad[:])

            def bias_tile(name, src, n):
                t = cp.tile([n, 1], f32, name=name)
                bs = wp.tile([n, 1], f32, name=f"bs_{name}", tag="wstg2")
                nc.sync.dma_start(bs[:], src.rearrange("(p o) -> p o", o=1))
                nc.scalar.copy(t[:], bs[:])
                return t

            gbz_e = bias_tile("gbz_e", egb_d[0:HID], HID)
            gbr_e = bias_tile("gbr_e", egb_d[HID : 2 * HID], HID)
            ub_e = bias_tile("ub_e", eub_d, HID)
            gbz_d = bias_tile("gbz_d", dgb_d[0:HID], HID)
            gbr_d = bias_tile("gbr_d", dgb_d[HID : 2 * HID], HID)
            ub_d = bias_tile("ub_d", dub_d, HID)
            pb = bias_tile("pb", pb_d, 1)
            hb = bias_tile("hb", hb_d, EMB)
            pw = cp.tile([HID, 1], f32)
            pws = wp.tile([HID, 1], f32, tag="wstg2")
            nc.sync.dma_start(pws[:], pw_d[:, :])
            nc.scalar.copy(rr(pw[:]), pws[:])
            hwt = cp.tile([HID, EMB], f32)
            hws = wp.tile([HID, EMB], f32, tag="wstg2")
            nc.sync.dma_start(hws[:], hw_d[:, :])
            nc.scalar.copy(rr(hwt[:]), hws[:])

            # ---- persistent per-batch state ---------------------------------
            zh64 = wp.tile([HID, N], f32, tag="wstg3")
            nc.vector.memset(zh64[:], 0.0)
            inp_pp = []
            cand_t = []
            ystg = []
            for b in range(BL):
                pair = []
                for p in range(2):
                    it = sp.tile([CIN, N], f32, name=f"inp{b}_{p}")
                    if p == 0:
                        nc.vector.tensor_copy(rr(it[0:HID, :]), zh64[:])
                    pair.append(it)
                inp_pp.append(pair)
                ct = sp.tile([CIN, N], f32, name=f"cand{b}")
                cand_t.append(ct)
                yt = sp.tile([CIN, N], f32, name=f"ystg{b}")
                ystg.append(yt)

            # ---- support construction (n-way interleaved) -------------------
            def build_supports(builds, tag):
                # builds: list of (emit_scores, at_tiles, at2_tiles)
                anm = {}
                for i in range(NT):
                    for bi, (es, _, _) in enumerate(builds):
                        ps = es(i)
                        nc.vector.tensor_scalar_max(ps[:], ps[:], 0.0)
                        ex = wp.tile(
                            [128, N], f32, name=f"ex_{tag}{bi}_{i}", tag="anm", bufs=8
                        )
                        esum = wp.tile(
                            [128, 1], f32, name=f"es_{tag}{bi}_{i}", tag="esum", bufs=4
                        )
                        nc.scalar.activation(
                            rr(ex[:]), ps[:], Act.Exp, accum_out=esum[:]
                        )
                        rinv = wp.tile(
                            [128, 1], f32, name=f"ri_{tag}{bi}_{i}", tag="rinv", bufs=4
                        )
                        nc.vector.reciprocal(rinv[:], esum[:])
                        nc.vector.tensor_scalar_mul(rr(ex[:]), ex[:], rinv[:])
                        anm[bi, i] = ex
                for bi, (_, at, _) in enumerate(builds):
                    for j in range(NT):
                        ps_t = acc_tile(f"ps_t_{tag}{bi}_{j}")
                        for i in range(NT):
                            nc.tensor.matmul(
                                ps_t[:, i * 128 : (i + 1) * 128],
                                anm[bi, i][:, j * 128 : (j + 1) * 128],
                                ident[:],
                                is_transpose=True,
                                skip_group_check=True,
                            )
                        nc.scalar.copy(rr(at[j][:]), ps_t[:])
                for bi, (_, at, at2) in enumerate(builds):
                    for j in range(NT):
                        ps_c = acc_tile(f"ps_c_{tag}{bi}_{j}")
                        for k in range(NT):
                            mm(
                                ps_c[:],
                                anm[bi, k][:, j * 128 : (j + 1) * 128],
                                at[k][:],
                                start=(k == 0),
                                stop=(k == NT - 1),
                            )
                        nc.scalar.mul(rr(at2[j][:]), ps_c[:], 2.0)
                        nc.vector.tensor_sub(
                            rr(at2[j][:, j * 128 : (j + 1) * 128]),
                            at2[j][:, j * 128 : (j + 1) * 128],
                            ident[:],
                        )

            # ---- encoder support --------------------------------------------
            ne_nm = []
            for i in range(NT):
                t = wp.tile([128, EMB], f32, name=f"ne_nm{i}", tag="ne_nm", bufs=NT)
                nc.sync.dma_start(t[:], ne_d[i * 128 : (i + 1) * 128, :])
                ne_nm.append(t)
            ps_ne = acc_tile("ps_ne", p=EMB)
            for i in range(NT):
                nc.tensor.matmul(
                    ps_ne[:, i * 128 : (i + 1) * 128],
                    ne_nm[i][:],
                    ident[:],
                    is_transpose=True,
                    skip_group_check=True,
                )
            neT = cp.tile([EMB, N], f32)
            nc.vector.tensor_copy(rr(neT[:]), ps_ne[:])

            aet = [cp.tile([128, N], f32, name=f"aet{j}") for j in range(NT)]
            aet2 = [cp.tile([128, N], f32, name=f"aet2_{j}") for j in range(NT)]

            def enc_scores(i):
                ps = acc_tile(f"ps_enc_s{i}")
                mm(ps[:], neT[:, i * 128 : (i + 1) * 128], neT[:], start=True, stop=True)
                return ps

            build_supports([(enc_scores, aet, aet2)], "enc")

            # ---- cell stage emitters ----------------------------------------
            # vsb layout [128, 1024]: block j*256 -> [v1_j(128) | v2_j(128)]
            # vcsb layout [128, 512]: block j*128 -> [vc1_j(64) | vc2_j(64)]
            def emit_vbuild(b, inp, gws, tag):
                ph = []
                for h in range(2):
                    ps = acc_tile(f"vps_{tag}{b}_{h}")
                    for jj in range(2):
                        j = 2 * h + jj
                        mm(
                            ps[:, jj * 256 : (jj + 1) * 256],
                            inp[:, j * 128 : (j + 1) * 128],
                            gws[:],
                            start=True,
                            stop=True,
                            skip_group_check=True,
                        )
                    ph.append(ps)
                vsb = wp.tile([128, 4 * 256], f32, name=f"vsb_{tag}{b}", tag="vsb", bufs=3)
                nc.scalar.copy(rr(vsb[:, 0:512]), ph[0][:])
                nc.scalar.copy(rr(vsb[:, 512:1024]), ph[1][:])
                return vsb

            def emit_zr(b, inp, vsb, gw0, at, at2, tag):
                zps = acc_tile(f"zps_{tag}{b}")
                mm(zps[:], gw0[:], inp[:], start=True, stop=False)
                for j in range(NT):
                    mm(
                        zps[:],
                        vsb[:, j * 256 : j * 256 + 128],
                        at[j][:],
                        start=False,
                        stop=False,
                    )
                for j in range(NT):
                    mm(
                        zps[:],
                        vsb[:, j * 256 + 128 : (j + 1) * 256],
                        at2[j][:],
                        start=False,
                        stop=(j == NT - 1),
                    )
                return zps

            def emit_vcbuild(b, cand, uws, tag):
                ph = []
                for h in range(2):
                    ps = acc_tile(f"vcps_{tag}{b}_{h}")
                    for jj in range(2):
                        j = 2 * h + jj
                        mm(
                            ps[:, jj * 256 : (jj + 1) * 256],
                            cand[:, j * 128 : (j + 1) * 128],
                            uws[:],
                            start=True,
                            stop=True,
                            skip_group_check=True,
                        )
                    ph.append(ps)
                vcsb = wp.tile([128, 512], f32, name=f"vcsb_{tag}{b}", tag="vcsb", bufs=4)
                for h in range(2):
                    src = ph[h].rearrange("p (j y s c) -> p j y s c", j=2, y=2, s=2, c=64)[
                        :, :, 0, :, :
                    ]
                    dst = vcsb[:, h * 256 : (h + 1) * 256].rearrange(
                        "p (j s c) -> p j s c", j=2, s=2, c=64
                    )
                    nc.scalar.copy(rr(dst), src)
                return vcsb

            def emit_hc(b, cand, vcsb, uw0, at, at2, tag):
                hps = acc_tile(f"hps_{tag}{b}", p=HID)
                mm(hps[:], uw0[:], cand[:], start=True, stop=False)
                for j in range(NT):
                    mm(
                        hps[:],
                        vcsb[:, j * 128 : j * 128 + 64],
                        at[j][:],
                        start=False,
                        stop=False,
                    )
                for j in range(NT):
                    mm(
                        hps[:],
                        vcsb[:, j * 128 + 64 : (j + 1) * 128],
                        at2[j][:],
                        start=False,
                        stop=(j == NT - 1),
                    )
                return hps

            def emit_gates(b, zps, gbz, gbr, tag):
                zt = wp.tile([HID, N], f32, name=f"z_{tag}{b}", tag="zsb", bufs=4)
                nc.scalar.activation(zt[:], zps[0:HID, :], Act.Sigmoid, bias=gbz[:])
                rt = wp.tile([HID, N], f32, name=f"r_{tag}{b}", tag="rsb", bufs=4)
                nc.scalar.activation(
                    rt[:], zps[HID : 2 * HID, :], Act.Sigmoid, bias=gbr[:]
                )
                return zt, rt

            def emit_cand_h(b, zt, inp, cand, tag):
                nc.vector.tensor_mul(rr(cand[0:HID, :]), zt[:], inp[0:HID, :])

            def emit_update(b, rt, hps, inp, nxt, ub, tag):
                hct = wp.tile([HID, N], f32, name=f"hc_{tag}{b}", tag="hct", bufs=3)
                nc.scalar.activation(hct[:], hps[:], Act.Tanh, bias=ub[:])
                dt = wp.tile([HID, N], f32, name=f"d_{tag}{b}", tag="dt", bufs=3)
                nc.vector.tensor_sub(dt[:], inp[0:HID, :], hct[:])
                nc.vector.tensor_mul(dt[:], rt[:], dt[:])
                nc.vector.tensor_add(rr(nxt[0:HID, :]), hct[:], dt[:])

            # interleaved A/C phase: vb0 vb1 zr0 vb2 zr1 vb3 zr2 zr3
            def phase_AC(curs, gws, gw0, ats, at2s, tag):
                vsbs = [None] * BL
                zps = [None] * BL
                order = [(0, "v"), (1, "v"), (0, "z"), (2, "v"), (1, "z"), (3, "v"),
                        (2, "z"), (3, "z")]
                for b, kind in order:
                    if kind == "v":
                        vsbs[b] = emit_vbuild(b, curs[b], gws, tag)
                    else:
                        zps[b] = emit_zr(
                            b, curs[b], vsbs[b], gw0, ats[b], at2s[b], tag
                        )
                return vsbs, zps

            def phase_FH(cands, uws, uw0, ats, at2s, tag):
                vcsbs = [None] * BL
                hps = [None] * BL
                order = [(0, "v"), (1, "v"), (0, "h"), (2, "v"), (1, "h"), (3, "v"),
                        (2, "h"), (3, "h")]
                for b, kind in order:
                    if kind == "v":
                        vcsbs[b] = emit_vcbuild(b, cands[b], uws, tag)
                    else:
                        hps[b] = emit_hc(
                            b, cands[b], vcsbs[b], uw0, ats[b], at2s[b], tag
                        )
                return vcsbs, hps

            # ---- encoder ----------------------------------------------------
            # prime xt for t=0 (staging rows 64:66 so copies are lane-aligned)
            for b in range(BL):
                xs = wp.tile([CIN, N], f32, name=f"xs0_{b}", tag="xs", bufs=4)
                nc.sync.dma_start(xs[HID:CIN, :], x_d[b, 0].rearrange("n c -> c n"))
                nc.gpsimd.tensor_copy(
                    rr(inp_pp[b][0][HID:CIN, :]), xs[HID:CIN, :]
                )

            aets = [aet] * BL
            aet2s = [aet2] * BL
            for t in range(T):
                curs = [inp_pp[b][t % 2] for b in range(BL)]
                nxts = [inp_pp[b][(t + 1) % 2] for b in range(BL)]
                tag = f"e{t}"
                # prefetch + stage xt(t+1) into nxt rows 64:66
                if t + 1 < T:
                    for b in range(BL):
                        xs = wp.tile(
                            [CIN, N], f32, name=f"xs{t+1}_{b}", tag="xs", bufs=4
                        )
                        nc.sync.dma_start(
                            xs[HID:CIN, :], x_d[b, t + 1].rearrange("n c -> c n")
                        )
                        nc.gpsimd.tensor_copy(
                            rr(nxts[b][HID:CIN, :]), xs[HID:CIN, :]
                        )
                vsbs, zps = phase_AC(curs, gws_e, gw0_e, aets, aet2s, tag)
                gates = [emit_gates(b, zps[b], gbz_e, gbr_e, tag) for b in range(BL)]
                for b in range(BL):
                    emit_cand_h(b, gates[b][0], curs[b], cand_t[b], tag)
                    nc.gpsimd.tensor_copy(
                        rr(cand_t[b][HID:CIN, :]), curs[b][HID:CIN, :]
                    )
                vcsbs, hps = phase_FH(cand_t, uws_e, uw0_e, aets, aet2s, tag)
                for b in range(BL):
                    emit_update(b, gates[b][1], hps[b], curs[b], nxts[b], ub_e, tag)

            # ---- decoder supports (hyper-network), 2-way interleaved --------
            adt = [
                [cp.tile([128, N], f32, name=f"adt{b}_{j}") for j in range(NT)]
                for b in range(BL)
            ]
            adt2 = [
                [cp.tile([128, N], f32, name=f"adt2_{b}_{j}") for j in range(NT)]
                for b in range(BL)
            ]
            h_fin = [inp_pp[b][T % 2] for b in range(BL)]
            for g in range(2):
                builds = []
                for b in (2 * g, 2 * g + 1):
                    ps_h = acc_tile(f"ps_hyp{b}", p=EMB)
                    mm(ps_h[:], hwt[:], h_fin[b][0:HID, :], start=True, stop=True)
                    neb = wp.tile([EMB, N], f32, name=f"neb{b}", tag="neb", bufs=2)
                    nc.scalar.activation(rr(neb[:]), ps_h[:], Act.Identity, bias=hb[:])

                    def dec_scores(i, neb=neb, b=b):
                        ps = acc_tile(f"ps_dec_s{b}_{i}")
                        mm(
                            ps[:],
                            neb[:, i * 128 : (i + 1) * 128],
                            neb[:],
                            start=True,
                            stop=True,
                        )
                        return ps

                    builds.append((dec_scores, adt[b], adt2[b]))
                build_supports(builds, f"dec{g}")

            # ---- decoder ----------------------------------------------------
            for b in range(BL):
                nc.vector.memset(ystg[b][HID : HID + 1, :], 0.0)  # go_0 = 0
            adts = [adt[b] for b in range(BL)]
            adt2s = [adt2[b] for b in range(BL)]
            for t in range(T):
                curs = [inp_pp[b][(T + t) % 2] for b in range(BL)]
                nxts = [inp_pp[b][(T + t + 1) % 2] for b in range(BL)]
                tag = f"d{t}"
                for b in range(BL):
                    nc.sync.dma_start(
                        ystg[b][HID + 1 : CIN, :], y_d[b, t].rearrange("n c -> c n")
                    )
                    nc.gpsimd.tensor_copy(
                        rr(curs[b][HID:CIN, :]), ystg[b][HID:CIN, :]
                    )
                    nc.gpsimd.tensor_copy(
                        rr(cand_t[b][HID:CIN, :]), ystg[b][HID:CIN, :]
                    )
                vsbs, zps = phase_AC(curs, gws_d, gw0_d, adts, adt2s, tag)
                gates = [
                    emit_gates(b, zps[b], gbz_d, gbr_d, tag) for b in range(BL)
                ]
                for b in range(BL):
                    emit_cand_h(b, gates[b][0], curs[b], cand_t[b], tag)
                vcsbs, hps = phase_FH(cand_t, uws_d, uw0_d, adts, adt2s, tag)
                for b in range(BL):
                    emit_update(b, gates[b][1], hps[b], curs[b], nxts[b], ub_d, tag)
                for b in range(BL):
                    psg = acc_tile(f"psg_{tag}{b}", p=1)
                    mm(psg[:], pw[:], nxts[b][0:HID, :], start=True, stop=True)
                    nc.scalar.activation(
                        ystg[b][HID : HID + 1, :], psg[:], Act.Identity, bias=pb[:]
                    )
                    nc.sync.dma_start(
                        out_d[b, t].rearrange("n c -> c n"),
                        ystg[b][HID : HID + 1, :],
                    )

    nc.compile()
    return nc


def _get_module():
    if "nc" not in _CACHE:
        _CACHE["nc"] = _build_module()
    return _CACHE["nc"]


def _in_maps(inputs):
    shared = {
        k: np.ascontiguousarray(np.asarray(inputs[k], dtype=np.float32))
        for k in (
            "node_emb",
            "enc_gW",
            "enc_gb",
            "enc_uW",
            "enc_ub",
            "dec_gW",
            "dec_gb",
            "dec_uW",
            "dec_ub",
            "proj_W",
            "proj_b",
            "hyper_W",
            "hyper_b",
        )
    }
    x = np.ascontiguousarray(np.asarray(inputs["x"], dtype=np.float32))
    y = np.ascontiguousarray(np.asarray(inputs["y_cov"], dtype=np.float32))
    maps = []
    for c in range(NCORES):
        m = dict(shared)
        m["x"] = np.ascontiguousarray(x[c * BL : (c + 1) * BL])
        m["y_cov"] = np.ascontiguousarray(y[c * BL : (c + 1) * BL])
        maps.append(m)
    return maps


def kernel(**inputs) -> np.ndarray:
    from concourse.bass_utils import run_bass_kernel_spmd

    nc = _get_module()
    maps = _in_maps(inputs)
    res = run_bass_kernel_spmd(nc, maps, list(range(NCORES)))
    out = np.concatenate([res.results[c]["out"] for c in range(NCORES)], axis=0)
    return out.astype(np.float32)
